# revision 1
# baseline (speedup 1.0000x reference)
"""Trainium2 Bass kernel for the gnn_message_passing problem.

Math (per edge e, side i):
  node_feat = l2norm(|dt|*w_time + b_time + gc*w_node + b_node)
  neigh_feat likewise per neighbor k
  att = tanh(node_feat@Wq + neigh_feat@Wk) . v_att
  score = leaky_relu(att + 2/(2+dt_neigh), 0.01)
  agg = sum_k (score*mask/n_neigh) * neigh_feat
  combined = [node_feat, agg]
  feat = sum_w exp(-0.5*bank_dt)*bank_mask * bank_feat + combined
  out = relu(feat @ weight.T)

Key structure exploited: every featurized vector lies in span{w_time, w_node,
b_time+b_node}, so node/neigh features are 3 scalars each. q+kk collapses to a
rank-6 combination of 6 fixed D-vectors; the "combined @ W.T" part of the
output collapses to a rank-6 combination of 6 fixed H-vectors. Only the
tanh( . ) . v contraction (E*2*K*D tanh evals) and the bank-feature reduction
touch O(E*K*D)-sized data on-device.

All heavy PE operands are bf16 (4x matmul rate, half the bank DMA bytes).
Coefficient scalars live in one strided SBUF tile (3 blocks of 272 cols:
256 neighbor + 8 self + 8 aggregate) so the self/aggregate rank-6 vectors
for any position tile are a single strided-view PE transpose away.

Sharding: pure data-parallel over E across 8 cores (one SPMD program).
"""

import numpy as np
import ml_dtypes

import concourse.bass as bass
import concourse.bacc as bacc
import concourse.mybir as mybir
import concourse.tile as tile
from concourse.bass_utils import run_bass_kernel_spmd

F32 = mybir.dt.float32
BF16 = mybir.dt.bfloat16
AF = mybir.ActivationFunctionType
OP = mybir.AluOpType

E, K, W, D, H = 4096, 32, 8, 128, 256
NCORES = 8
EC = E // NCORES          # 512 edges per core
POS = EC * 2              # 1024 (edge, side) positions per core
NT = POS // 128           # 8 position tiles of 128
D2 = 2 * D                # 256
CHUNKS = 4                # tanh chunks of 1024 cols per tile
BW = 272                  # abg block width: 256 neigh + 8 self + 8 agg


def _build_program(pp):
    """Build the SPMD single-core program. pp: dict of host-precomputed params."""
    nc = bacc.Bacc("TRN2", target_bir_lowering=False, debug=False)

    # ---- DRAM I/O (per core shard), host-prepermuted layouts ----
    d_ap = nc.dram_tensor("a_p", [128, 264], F32, kind="ExternalInput")
    d_bp = nc.dram_tensor("b_p", [128, 264], F32, kind="ExternalInput")
    d_msk = nc.dram_tensor("mskn_p", [128, 256], F32, kind="ExternalInput")
    d_bdt = nc.dram_tensor("bdt_e", [128, 64], F32, kind="ExternalInput")
    d_bmsk = nc.dram_tensor("bmsk_e", [128, 64], F32, kind="ExternalInput")
    # chunk-contiguous bf16: chunk c=(t*4+j)*2+wh -> rows c*128..(c+1)*128
    d_bft = nc.dram_tensor("bft_p", [64 * 128, D2], BF16, kind="ExternalInput")
    d_out = nc.dram_tensor("out", [POS, H], F32, kind="ExternalOutput")

    # ---- inline constants ----
    c_basis = nc.inline_tensor(pp["basis6att4"], name="c_basis")   # [128,128] bf16
    c_b6h = nc.inline_tensor(pp["basis6H"], name="c_b6h")          # [6,256] bf16
    c_v = nc.inline_tensor(pp["v32"], name="c_v")                  # [128,32] bf16
    c_wT = nc.inline_tensor(pp["weightT"], name="c_wT")            # [256,256] bf16
    c_dmask = nc.inline_tensor(pp["dmask"], name="c_dmask")        # [128,32] f32
    c_ident = nc.inline_tensor(pp["ident"], name="c_ident")        # [128,128] f32
    c_eps = nc.inline_tensor(np.full((128, 1), 1e-24, np.float32), name="c_eps")
    c_identb = nc.inline_tensor(pp["ident"].astype(ml_dtypes.bfloat16), name="c_identb")
    G = pp["gram"]  # 3x3 float

    from contextlib import ExitStack
    with tile.TileContext(nc) as tc, ExitStack() as ctx:
        cpool = ctx.enter_context(tc.tile_pool(name="consts", bufs=1))
        wpool = ctx.enter_context(tc.tile_pool(name="work", bufs=1))
        p_coef6 = ctx.enter_context(tc.tile_pool(name="coef6", bufs=3))
        p_tanh = ctx.enter_context(tc.tile_pool(name="tanh", bufs=6))
        p_attT = ctx.enter_context(tc.tile_pool(name="attT", bufs=3))
        p_featT = ctx.enter_context(tc.tile_pool(name="featT", bufs=4))
        p_pft = ctx.enter_context(tc.tile_pool(name="pft", bufs=2))
        p_bch = ctx.enter_context(tc.tile_pool(name="bch", bufs=4))
        p_mblk = ctx.enter_context(tc.tile_pool(name="mblk", bufs=4))
        p_mb8 = ctx.enter_context(tc.tile_pool(name="mb8", bufs=8))
        p_bankC = ctx.enter_context(tc.tile_pool(name="bankC", bufs=8))
        p_out = ctx.enter_context(tc.tile_pool(name="outp", bufs=2))
        p_ast = ctx.enter_context(tc.tile_pool(name="ast", bufs=2))
        ps_arg = ctx.enter_context(tc.tile_pool(name="ps_arg", bufs=2, space="PSUM"))
        ps_pv = ctx.enter_context(tc.tile_pool(name="ps_pv", bufs=2, space="PSUM"))
        ps_mix = ctx.enter_context(tc.tile_pool(name="ps_mix", bufs=2, space="PSUM"))

        # ---- input loads: SP queue carries the DVE-feed tensors, Pool queue
        # the bank-side tensors and most constants ----
        a_all = wpool.tile([128, 264], F32, name="a_all")
        b_all = wpool.tile([128, 264], F32, name="b_all")
        nc.sync.dma_start(out=a_all, in_=d_ap[:, :])
        nc.sync.dma_start(out=b_all, in_=d_bp[:, :])
        cb_basis = cpool.tile([128, 128], BF16, name="cb_basis")
        nc.sync.dma_start(out=cb_basis, in_=c_basis[:, :])
        cb_id = cpool.tile([128, 128], F32, name="cb_id")
        nc.sync.dma_start(out=cb_id, in_=c_ident[:, :])
        cb_eps = cpool.tile([128, 1], F32, name="cb_eps")
        nc.sync.dma_start(out=cb_eps, in_=c_eps[:, :])
        cb_idb = cpool.tile([128, 128], BF16, name="cb_idb")
        nc.sync.dma_start(out=cb_idb, in_=c_identb[:, :])
        t_m = wpool.tile([128, 256], F32, name="t_m")
        nc.sync.dma_start(out=t_m[:, :], in_=d_msk[:, :])

        cb_v = cpool.tile([128, 32], BF16, name="cb_v")
        nc.gpsimd.dma_start(out=cb_v, in_=c_v[:, :])
        cb_b6h = cpool.tile([6, 256], BF16, name="cb_b6h")
        nc.gpsimd.dma_start(out=cb_b6h, in_=c_b6h[:, :])
        cb_wT0 = cpool.tile([128, 256], BF16, name="cb_wT0")
        nc.gpsimd.dma_start(out=cb_wT0, in_=c_wT[0:128, :])
        cb_wT1 = cpool.tile([128, 256], BF16, name="cb_wT1")
        nc.gpsimd.dma_start(out=cb_wT1, in_=c_wT[128:256, :])
        cb_dmask = cpool.tile([128, 32], F32, name="cb_dmask")
        nc.gpsimd.dma_start(out=cb_dmask, in_=c_dmask[:, :])

        # ---- bank chunk loads: one DMA per tile, 8 chunks each ----
        def load_bank_tile(t):
            bc = p_bch.tile([128, 8 * D2], BF16, tag="bch", name=f"bc_{t}")
            nc.gpsimd.dma_start(
                out=bc.rearrange("p (g d) -> p g d", d=D2),
                in_=d_bft[t * 1024:(t + 1) * 1024, :].rearrange(
                    "(g p) d -> p g d", p=128))
            return bc

        bc_tiles = {}
        for t in range(3):
            bc_tiles[t] = load_bank_tile(t)
        # loaded late so the ACT exp becomes ready after the absrsqrt: one
        # exp_and_others table load then covers exp + all tanh
        bdt_e = wpool.tile([128, 64], F32, name="bdt_e")
        bmsk_e = wpool.tile([128, 64], F32, name="bmsk_e")
        nc.gpsimd.dma_start(out=bdt_e, in_=d_bdt[:, :])
        nc.gpsimd.dma_start(out=bmsk_e, in_=d_bmsk[:, :])

        # ---- featurize scalars (DVE; shortest chain to the coef tile) ----
        # abg blocks of 272 cols: [alpha | beta | invn], each
        # [256 neigh | 8 self | 8 agg]
        abg = wpool.tile([128, 3 * BW], F32, name="abg")
        AL, BE, IV = 0 * BW, 1 * BW, 2 * BW
        aab = wpool.tile([128, 264], F32, name="aab")
        nc.vector.scalar_tensor_tensor(out=aab, in0=a_all, scalar=-1.0,
                                       in1=a_all, op0=OP.mult, op1=OP.max)
        bb = wpool.tile([128, 264], F32, name="bb")
        nc.vector.tensor_tensor(out=bb, in0=b_all, in1=b_all, op=OP.mult)
        n2 = wpool.tile([128, 264], F32, name="n2")
        # n2 = a*(G00*a + 2*G01*b + 2*G02) + G11*b^2 + 2*G12*b + G22
        nc.vector.tensor_scalar(out=n2, in0=aab, scalar1=float(G[0, 0]),
                                scalar2=float(2 * G[0, 2]), op0=OP.mult,
                                op1=OP.add)
        nc.vector.scalar_tensor_tensor(out=n2, in0=b_all,
                                       scalar=float(2 * G[0, 1]),
                                       in1=n2, op0=OP.mult, op1=OP.add)
        nc.vector.tensor_tensor(out=n2, in0=n2, in1=aab, op=OP.mult)
        nc.vector.scalar_tensor_tensor(out=n2, in0=bb, scalar=float(G[1, 1]),
                                       in1=n2, op0=OP.mult, op1=OP.add)
        if abs(G[1, 2]) > 1e-30 or abs(G[2, 2]) > 1e-30:
            nc.vector.tensor_scalar(out=n2, in0=b_all,
                                    scalar1=float(2 * G[1, 2]),
                                    scalar2=float(G[2, 2]), op0=OP.mult,
                                    op1=OP.add)
        # invn = 1/sqrt(n2 + 1e-24) = 1/max(norm, 1e-12); eps via ACT bias
        nc.scalar.activation(out=abg[:, IV:IV + 264], in_=n2,
                             func=AF.Abs_reciprocal_sqrt, bias=cb_eps[:, :])
        nc.vector.tensor_tensor(out=abg[:, AL:AL + 264], in0=aab,
                                in1=abg[:, IV:IV + 264], op=OP.mult)
        nc.vector.tensor_tensor(out=abg[:, BE:BE + 264], in0=b_all,
                                in1=abg[:, IV:IV + 264], op=OP.mult)

        att_a = wpool.tile([128, 256], F32, name="att_a")
        bankC_sb = [None] * NT

        # ---- self coef transpose: one strided gather + one transpose ----
        # selfT rows c*8+t (c in alpha,beta,invn)
        packS = wpool.tile([128, 24], F32, name="packS")
        nc.vector.tensor_copy(
            out=packS.rearrange("p (c x) -> p c x", x=8),
            in_=abg.rearrange("p (c x) -> p c x", x=BW)[:, :, 256:264])
        pmS = ps_mix.tile([128, 256], F32, tag="mix", name="pm_selfT")
        nc.tensor.transpose(pmS[0:24, 0:128], packS, cb_id)
        selfT = wpool.tile([32, 128], BF16, name="selfT")
        nc.vector.tensor_copy(out=selfT[0:24, :], in_=pmS[0:24, 0:128])

        # neighbor coef transposes: coefT_h[h] rows=(t-within-half, k), 128 pos
        coefT_h = [wpool.tile([128, 384], BF16, name=f"coefTh{h}")
                   for h in range(2)]
        for h in range(2):
            for ci, coff in enumerate((AL, BE, IV)):
                pmx = ps_mix.tile([128, 256], F32, tag="mix",
                                  name=f"pm_{ci}{h}")
                nc.tensor.transpose(
                    pmx[0:128, 0:128],
                    abg[:, coff + h * 128:coff + (h + 1) * 128], cb_id)
                nc.vector.tensor_copy(
                    out=coefT_h[h][:, ci * 128:(ci + 1) * 128],
                    in_=pmx[0:128, 0:128])

        # ---- bank decay weights (ACT exp shares the exp_and_others table
        # with every later tanh) + score-side DVE prep ----
        bwe = wpool.tile([128, 64], F32, name="bwe")
        nc.scalar.activation(out=bwe, in_=bdt_e, func=AF.Exp, scale=-0.5)
        nc.vector.tensor_tensor(out=bwe, in0=bwe, in1=bmsk_e, op=OP.mult)

        # time decay 2/(2+dt) on raw dt
        ts_t = wpool.tile([128, 256], F32, name="ts_t")
        nc.vector.tensor_scalar(out=ts_t, in0=a_all[:, 0:256], scalar1=2.0,
                                scalar2=None, op0=OP.add)
        nc.vector.reciprocal_approx_fast(out=ts_t, in_=ts_t)
        nc.vector.tensor_scalar(out=ts_t, in0=ts_t, scalar1=2.0, scalar2=None,
                                op0=OP.mult)

        # n_neigh and mask/n_neigh
        nn = wpool.tile([128, 8], F32, name="nn")
        nc.vector.tensor_reduce(out=nn, in_=t_m.rearrange("p (t k) -> p t k", k=K),
                                axis=mybir.AxisListType.X, op=OP.add)
        nc.vector.tensor_scalar(out=nn, in0=nn, scalar1=1.0, scalar2=None,
                                op0=OP.max)
        innn = wpool.tile([128, 8], F32, name="innn")
        nc.vector.reciprocal_approx_fast(out=innn, in_=nn)
        mrec = wpool.tile([128, 256], F32, name="mrec")
        nc.vector.tensor_tensor(
            out=mrec.rearrange("p (t k) -> p t k", k=K),
            in0=t_m.rearrange("p (t k) -> p t k", k=K),
            in1=innn.unsqueeze(2).broadcast_to([128, 8, K]), op=OP.mult)

        # ---- per-2-tile coef6 build (rhs for the arg matmuls) ----
        def build_coef6_grp(g):
            t0 = 2 * g
            c6 = p_coef6.tile([6, 2 * CHUNKS * 1024], BF16, tag="coef6",
                              name=f"coef6_{g}")
            ch = coefT_h[t0 // 4]
            r0 = (t0 % 4) * 32
            for c in range(3):
                nc.sync.dma_start(
                    out=c6[c:c + 1, :],
                    in_=selfT[c * 8 + t0:c * 8 + t0 + 2, :]
                    .unsqueeze(1).broadcast_to([2, K, 128]))
            for c in range(3):
                nc.sync.dma_start(
                    out=c6[3 + c:4 + c, :],
                    in_=ch[r0:r0 + 64, c * 128:(c + 1) * 128])
            return c6

        # ---- bank decay-weight blocks, all up-front on the Pool engine ----
        mb_all = []
        for t in range(NT):
            mb = p_mb8.tile([128, 256], BF16, tag="mblk", name=f"mb_{t}")
            nc.gpsimd.tensor_tensor(
                out=mb.rearrange("r (b c) -> r b c", c=32),
                in0=cb_dmask.unsqueeze(1).broadcast_to([128, 8, 32]),
                in1=bwe[:, t * 8:(t + 1) * 8].unsqueeze(2).broadcast_to(
                    [128, 8, 32]),
                op=OP.mult)
            mb_all.append(mb)

        close_a_q = []
        close_b_q = []

        def bank_mms(tb):
            mb = mb_all[tb]
            bc = bc_tiles.pop(tb)
            fpA = ps_mix.tile([128, 256], F32, tag="mix", name=f"fpA_{tb}")
            for wh in range(2):
                for j in range(4):
                    g = 2 * j + wh
                    nc.tensor.matmul(
                        fpA[32 * j:32 * (j + 1), 0:256],
                        lhsT=mb[:, 32 * g:32 * (g + 1)],
                        rhs=bc[:, g * D2:(g + 1) * D2],
                        start=(wh == 0), stop=(wh == 1),
                        skip_group_check=True,
                        tile_position=(0, 32 * j))
            bkA = p_mblk.tile([128, 256], BF16, tag="bkA", name=f"bkA_{tb}")
            nc.vector.tensor_copy(out=bkA, in_=fpA[:, 0:256])
            close_a_q.append((tb, bkA))

        def bank_close_a():
            # bkA is ~2 chunks old here: the XBAR transposes never block
            # the SP queue head
            if not close_a_q:
                return
            tb, bkA = close_a_q.pop(0)
            fsb = [None, None]
            for hh in range(2):
                fsb[hh] = p_featT.tile([128, 128], BF16, tag="featT",
                                       name=f"fT_{tb}_{hh}")
                nc.sync.dma_start_transpose(
                    out=fsb[hh], in_=bkA[:, hh * 128:(hh + 1) * 128])
            close_b_q.append((tb, fsb))

        def bank_close_b():
            if not close_b_q:
                return
            tb, fsb = close_b_q.pop(0)
            poB = ps_mix.tile([128, 256], F32, tag="mix", name=f"poB_{tb}")
            nc.tensor.matmul(poB[:, 0:256], lhsT=fsb[0], rhs=cb_wT0,
                             start=True, stop=False)
            nc.tensor.matmul(poB[:, 0:256], lhsT=fsb[1], rhs=cb_wT1,
                             start=False, stop=True)
            bankC_sb[tb] = p_bankC.tile([128, 256], F32, tag="bankC",
                                        name=f"bankC_{tb}")
            nc.vector.tensor_copy(out=bankC_sb[tb], in_=poB[:, 0:256])

        coef6_g = {0: build_coef6_grp(0)}

        # prologue: bank tiles 0-1 fill the PE while DVE computes the
        # attention coefficients
        for tb in range(2):
            if tb + 3 < NT:
                bc_tiles[tb + 3] = load_bank_tile(tb + 3)
            bank_mms(tb)

        state = {}
        pend = []               # [(th, cc, t)] vdots not yet emitted
        pend_pmxa = []          # att transposes delayed one chunk
        out_work = []           # deferred per-tile output closures

        def emit_vgroup(th0, th1, cc1, t):
            # 4 col-group matmuls back-to-back: concurrent in the PE array
            g = (t * CHUNKS + cc1) // 2
            pv = ps_pv.tile([128, 512], F32, tag="pv", name=f"pv_{g}")
            for q, (thx, mm) in enumerate(((th0, 0), (th0, 1),
                                           (th1, 0), (th1, 1))):
                nc.tensor.matmul(pv[32 * q:32 * (q + 1), :], lhsT=cb_v,
                                 rhs=thx[:, mm * 512:(mm + 1) * 512],
                                 start=True, stop=True,
                                 tile_position=(0, 32 * q))
            # stage PSUM->SBUF (DVE), then extract the 16 useful rows (one
            # per PE col group) with a strided DMA
            b = cc1 // 2
            if b == 0:
                state[t]["ast"] = p_ast.tile([128, 1024], BF16, tag="ast",
                                             name=f"ast_{t}")
            ast = state[t]["ast"]
            nc.vector.tensor_copy(out=ast[:, 512 * b:512 * (b + 1)], in_=pv)
            attT = state[t]["attT"]
            nc.gpsimd.dma_start(
                out=attT[16 * b:16 * (b + 1), :],
                in_=ast[:, 512 * b:512 * (b + 1)].rearrange(
                    "(q r) (kl p) -> q r kl p", r=32, p=128)[:, 0])
            if b == 1:
                pend_pmxa.append(t)

        sc = wpool.tile([128, 256], F32, name="sc")
        sc2 = wpool.tile([128, 256], F32, name="sc2")
        wgt = wpool.tile([128, 256], F32, name="wgt")
        prod = wpool.tile([128, 256], F32, name="prod")

        def emit_score_half(hh):
            s = slice(hh * 128, (hh + 1) * 128)
            nc.vector.tensor_tensor(out=sc[:, s], in0=att_a[:, s],
                                    in1=ts_t[:, s], op=OP.add)
            nc.vector.tensor_scalar(out=sc2[:, s], in0=sc[:, s], scalar1=0.01,
                                    scalar2=None, op0=OP.mult)
            nc.vector.tensor_tensor(out=sc[:, s], in0=sc[:, s], in1=sc2[:, s],
                                    op=OP.max)
            nc.vector.tensor_tensor(out=wgt[:, s], in0=sc[:, s],
                                    in1=mrec[:, s], op=OP.mult)
            for c, coff in enumerate((AL, BE, IV)):
                nc.vector.tensor_tensor(out=prod[:, s], in0=wgt[:, s],
                                        in1=abg[:, coff + hh * 128:
                                                coff + (hh + 1) * 128],
                                        op=OP.mult)
                nc.vector.tensor_reduce(
                    out=abg[:, coff + 264 + hh * 4:coff + 264 + (hh + 1) * 4],
                    in_=prod[:, s].rearrange("p (t k) -> p t k", k=K),
                    axis=mybir.AxisListType.X, op=OP.add)
            for t in range(4 * hh, 4 * hh + 4):
                out_work.append(lambda t=t: emit_out(t))

        def emit_out(t):
            # rank-6 coef vector for tile t: [128, (c 3), (node/agg 2)] view
            view = abg.rearrange("p (c x) -> p c x", x=BW)[:, :, 256:272]
            view = view.rearrange("p c (d e) -> p c d e", e=8)[:, :, :, t]
            packO = wpool.tile([128, 6], BF16, name=f"packO_{t}")
            nc.vector.tensor_copy(
                out=packO.rearrange("p (c d) -> p c d", d=2), in_=view)
            pq = ps_mix.tile([128, 256], BF16, tag="mix", name=f"pq_{t}")
            nc.tensor.transpose(pq[0:6, 0:128], packO, cb_idb)
            pft = p_pft.tile([6, 128], BF16, tag="pft", name=f"pft_{t}")
            nc.vector.tensor_copy(out=pft, in_=pq[0:6, 0:128])
            pc = ps_mix.tile([128, 256], F32, tag="mix", name=f"pc_{t}")
            nc.tensor.matmul(pc[:, 0:256], lhsT=pft, rhs=cb_b6h,
                             start=True, stop=True)
            ot = p_out.tile([128, 256], F32, tag="outp", name=f"ot_{t}")
            nc.vector.tensor_tensor(out=ot, in0=pc[:, 0:256], in1=bankC_sb[t],
                                    op=OP.add)
            nc.vector.tensor_scalar(out=ot, in0=ot, scalar1=0.0, scalar2=None,
                                    op0=OP.max)
            nc.gpsimd.dma_start(out=d_out[t * 128:(t + 1) * 128, :], in_=ot)

        def flush_pmxa():
            while pend_pmxa:
                tx = pend_pmxa.pop(0)
                attT = state[tx]["attT"]
                pmx = ps_mix.tile([128, 256], BF16, tag="mix", name=f"pmxa_{tx}")
                nc.tensor.transpose(pmx[0:128, 0:32], attT, cb_idb[0:32, 0:32])
                nc.vector.tensor_copy(out=att_a[:, 32 * tx:32 * (tx + 1)],
                                      in_=pmx[0:128, 0:32])
                if tx == 3:
                    emit_score_half(0)

        # ---- software-pipelined global chunk loop ----
        for gc in range(NT * CHUNKS):
            t, cc = divmod(gc, CHUNKS)
            tb = t + 2           # bank tile handled during this att tile
            if cc == 0:
                state[t] = {"attT": p_attT.tile([32, 128], BF16, tag="attT",
                                                name=f"attT_{t}")}
            c6 = coef6_g[t // 2]
            base = (t % 2) * (CHUNKS * 1024) + cc * 1024
            pa = ps_arg.tile([128, 1024], F32, tag="psarg", name=f"pa_{gc}")
            for mm in range(2):
                nc.tensor.matmul(
                    pa[:, mm * 512:(mm + 1) * 512], lhsT=cb_basis[0:6, :],
                    rhs=c6[:, base + mm * 512:base + (mm + 1) * 512],
                    start=True, stop=True)
            th = p_tanh.tile([128, 1024], BF16, tag="tanh", name=f"th_{gc}")
            nc.scalar.activation(out=th, in_=pa, func=AF.Tanh)
            if len(pend) >= 4 and pend[0][1] % 2 == 0:
                (th0, _, _), (th1, cc1, t1) = pend.pop(0), pend.pop(0)
                emit_vgroup(th0, th1, cc1, t1)
            pend.append((th, cc, t))
            if cc == 0 and tb < NT:
                if tb + 3 < NT:
                    bc_tiles[tb + 3] = load_bank_tile(tb + 3)
                bank_mms(tb)
            if cc == 1:
                bank_close_b()
            if cc == 2:
                if t % 2 == 0 and t + 2 < NT:
                    coef6_g[(t + 2) // 2] = build_coef6_grp((t + 2) // 2)
            if cc == 3:
                bank_close_a()
            flush_pmxa()
            if out_work:
                out_work.pop(0)()
        while pend:
            (th0, _, _), (th1, cc1, t1) = pend.pop(0), pend.pop(0)
            emit_vgroup(th0, th1, cc1, t1)
            flush_pmxa()
            if out_work:
                out_work.pop(0)()
        while close_a_q:
            bank_close_a()
        while close_b_q:
            bank_close_b()

        # ---- second-half scores + outputs ----
        emit_score_half(1)
        while out_work:
            out_work.pop(0)()

    nc.compile()
    return nc


def _host_params(w_time, b_time, w_node, b_node, Wq, Wk, v_att, weight):
    f32 = np.float32
    bf16 = ml_dtypes.bfloat16
    w_time = np.asarray(w_time, f32)
    w_node = np.asarray(w_node, f32)
    bsum = np.asarray(b_time, f32) + np.asarray(b_node, f32)
    Wq = np.asarray(Wq, f32)
    Wk = np.asarray(Wk, f32)
    v = np.asarray(v_att, f32)
    weight = np.asarray(weight, f32)

    basis3 = np.stack([w_time, w_node, bsum])                  # [3, D]
    gram = basis3 @ basis3.T
    basis6att = np.zeros((6, D), f32)
    basis6att[0:3] = basis3 @ Wq
    basis6att[3:6] = basis3 @ Wk
    basis6H = np.zeros((6, H), f32)
    basis6H[0:3] = basis3 @ weight[:, :D].T
    basis6H[3:6] = basis3 @ weight[:, D:].T
    # emit_out's transpose produces rows (alpha, A, beta, B, invn, C)
    basis6H = basis6H[[0, 3, 1, 4, 2, 5]]
    dmask = np.zeros((128, 32), f32)
    dmask[np.arange(128), np.arange(128) // 4] = 1.0
    basis6att4 = np.zeros((128, D), f32)
    for q in range(4):
        basis6att4[32 * q:32 * q + 6] = basis6att
    return {
        "basis6att4": basis6att4.astype(bf16),
        "basis6H": basis6H.astype(bf16),
        "v32": np.ascontiguousarray(np.tile(v.reshape(D, 1), (1, 32))).astype(bf16),
        "weightT": np.ascontiguousarray(weight.T).astype(bf16),
        "dmask": dmask,
        "ident": np.eye(128, dtype=f32),
        "gram": gram.astype(np.float64),
    }


def _perm_tk(x):
    # [EC,2,K] -> [128 p, (t k)]
    return np.ascontiguousarray(
        x.reshape(NT, 128, K).transpose(1, 0, 2).reshape(128, NT * K))


def _perm_t(x):
    # [EC,2] -> [128 p, t]
    return np.ascontiguousarray(x.reshape(NT, 128).T)


def _perm_bft(x):
    # [EC,2,W,D2] -> rows ((t j wh),(po wl)) x D2, bf16
    x = x.reshape(NT, 4, 32, 2, 4, D2)       # t j po wh wl d
    x = x.transpose(0, 1, 3, 2, 4, 5)        # t j wh po wl d
    return np.ascontiguousarray(
        x.reshape(64 * 128, D2).astype(ml_dtypes.bfloat16))


def _expand_bank(x):
    # [EC,2,W] -> [128 (po,wl), 64 (t,j,wh)]: x[t*128+j*32+po, wh*4+wl]
    x = x.reshape(NT, 4, 32, 2, 4)          # t j po wh wl
    x = x.transpose(2, 4, 0, 1, 3)          # po wl t j wh
    return np.ascontiguousarray(x.reshape(128, 64))


def _shard_inputs(inputs):
    f32 = np.float32
    ins = []
    for c in range(NCORES):
        sl = slice(c * EC, (c + 1) * EC)
        ins.append({
            "a_p": np.concatenate(
                [_perm_tk(np.asarray(inputs["dt_neigh"][sl], f32)),
                 _perm_t(np.asarray(inputs["dt_self"][sl], f32))], axis=1),
            "b_p": np.concatenate(
                [_perm_tk(np.asarray(inputs["gc_neigh"][sl], f32)),
                 _perm_t(np.asarray(inputs["gc_self"][sl], f32))], axis=1),
            "mskn_p": _perm_tk(
                np.asarray(inputs["neigh_mask"][sl]).astype(f32)),
            "bdt_e": _expand_bank(np.asarray(inputs["bank_dt"][sl], f32)),
            "bmsk_e": _expand_bank(
                np.asarray(inputs["bank_mask"][sl]).astype(f32)),
            "bft_p": _perm_bft(np.asarray(inputs["bank_feat"][sl], f32)),
        })
    return ins


_LAST_RESULT = {}


def kernel(**inputs):
    pp = _host_params(inputs["w_time"], inputs["b_time"], inputs["w_node"],
                      inputs["b_node"], inputs["Wq"], inputs["Wk"],
                      inputs["v_att"], inputs["weight"])
    nc = _build_program(pp)
    in_maps = _shard_inputs(inputs)
    import os
    trace = bool(int(os.environ.get("KBENCH_TRACE", "0")))
    res = run_bass_kernel_spmd(nc, in_maps, core_ids=list(range(NCORES)),
                               trace=trace)
    _LAST_RESULT["res"] = res
    outs = [res.results[c]["out"].reshape(EC, 2, H) for c in range(NCORES)]
    return np.ascontiguousarray(np.concatenate(outs, axis=0))

